# revision 1
# baseline (speedup 1.0000x reference)
"""Lorentz per-head causal attention on 8 trn2 NeuronCores.

Sharding: core c -> batch b=c//4, heads {2*(c%4), 2*(c%4)+1}.
W_q/W_k/W_v column-sharded, W_o row-sharded; host sums the 4 partial
outputs per batch (replaces the tensor-parallel AllReduce).

Per-core kernel (all compute in f32):
  A: log-map x -> x_eu, transposed into [D,S] layout via per-token-tile
     matmuls against diag(theta/nrm) (fuses the scaling with the transpose).
  B: QKV projection [S,384] (2 heads x Q,K,V); batched exp-map stats;
     assemble Lorentz-lifted Qt=[c*f*Q, c*t], Kt=[-f*K, t] in [65,S] layout
     via PE transposes. V kept token-major with a ones column appended so
     the PV matmul also produces the softmax denominator for free.
  C: per head, per 512-wide q block: scoresT[k,q] matmuls (K=65), exp on
     ACT over [128,1024] pairs, causal masks (multiplicative, host-built)
     on diagonal tiles only, PV accumulation in PSUM [65,512]; normalize
     by broadcasting 1/denom with a K=1 ones matmul.
  D: W_o row-shard matmul, DMA out.
Softmax skips max-subtraction: scores = abs_K*(qt*kt - qs.ks)/8 are O(1)
for these inputs (verified < 10), so exp cannot overflow.
"""
import sys

sys.path.insert(0, "/opt/trn_rl_repo")

from contextlib import ExitStack

import numpy as np

import concourse.bacc as bacc
import concourse.bass as bass
import concourse.mybir as mybir
from concourse.bass_utils import run_bass_kernel_spmd
from concourse.tile import TileContext

F32 = mybir.dt.float32
AF = mybir.ActivationFunctionType

B, S, D, H, DH = 2, 2048, 512, 8, 64
EPS = 1e-7
NT = S // 128  # 16 token tiles
NCORES = 8

_NC_CACHE = {}


def _emit_program():
    nc = bacc.Bacc(None)
    x_in = nc.declare_dram_parameter("x", [S, D + 1], F32, isOutput=False)
    wqkv_in = nc.declare_dram_parameter("wqkv", [D, 384], F32, isOutput=False)
    wo_in = nc.declare_dram_parameter("wo", [128, D], F32, isOutput=False)
    masks_in = nc.declare_dram_parameter("masks", [128, 2048], F32, isOutput=False)
    hc_in = nc.declare_dram_parameter("hconst", [128, 192], F32, isOutput=False)
    id_in = nc.declare_dram_parameter("ident", [128, 128], F32, isOutput=False)
    out_d = nc.declare_dram_parameter("out", [S, D], F32, isOutput=True)

    with TileContext(nc) as tc, ExitStack() as ctx:
        cpool = ctx.enter_context(tc.tile_pool(name="consts", bufs=1))
        ppool = ctx.enter_context(tc.tile_pool(name="persist", bufs=1))
        wpool = ctx.enter_context(tc.tile_pool(name="work", bufs=3))
        pspool = ctx.enter_context(tc.tile_pool(name="ps", bufs=2, space="PSUM"))

        # ---- constants ----
        wqkv = cpool.tile([128, 4 * 384], F32)
        for c in range(4):
            nc.gpsimd.dma_start(
                wqkv[:, c * 384:(c + 1) * 384], wqkv_in[c * 128:(c + 1) * 128, :]
            )
        wo_t = cpool.tile([128, 512], F32)
        nc.gpsimd.dma_start(wo_t[:], wo_in[:])
        maskt = cpool.tile([128, 2048], F32)
        nc.gpsimd.dma_start(maskt[:], masks_in[:])
        hc = cpool.tile([128, 192], F32)
        nc.gpsimd.dma_start(hc[:], hc_in[:])
        ident = cpool.tile([128, 128], F32)
        nc.gpsimd.dma_start(ident[:], id_in[:])
        ones64 = cpool.tile([1, 64], F32)
        nc.vector.memset(ones64[:], 1.0)

        # ---- persistent intermediates ----
        # x_euT, per-tt chunk layout: tile[tt%2][:, (tt//2)*512 + c*128]
        xeTa = ppool.tile([128, 8 * 512], F32)
        xeTb = ppool.tile([128, 8 * 512], F32)
        xeT = [xeTa, xeTb]
        # [Qt_h0 | Qt_h1 | Kt_h0 | Kt_h1], each [65, 2048]
        qkT = ppool.tile([65, 4 * 2048], F32)
        # V-hat per head: NT groups of 65 cols, col 64 stays 1.0
        vh = ppool.tile([128, 2 * NT * 65], F32)
        nc.gpsimd.memset(vh[:], 1.0)
        qkvN = ppool.tile([128, NT * 384], F32)
        outT = ppool.tile([128, 4 * 512], F32)
        sqall = ppool.tile([128, 2048], F32)
        ss_all = ppool.tile([128, 64], F32)
        n_all = ppool.tile([128, 64], F32)
        m_all = ppool.tile([128, 64], F32)
        e1_all = ppool.tile([128, 64], F32)
        e2_all = ppool.tile([128, 64], F32)
        u_all = ppool.tile([128, 64], F32)
        w_all = ppool.tile([128, 64], F32)
        rn_all = ppool.tile([128, 64], F32)
        g_all = ppool.tile([128, 64], F32)
        tv_all = ppool.tile([128, 64], F32)

        # ---- stage A: batched log-map stats ----
        xall = ppool.tile([128, NT * 513], F32)
        nc.gpsimd.dma_start(
            xall[:].rearrange("p (t c) -> p t c", c=513),
            x_in[:].rearrange("(t p) c -> p t c", p=128),
        )
        zA = ppool.tile([128, NT], F32)
        z2A = ppool.tile([128, NT], F32)
        rA = ppool.tile([128, NT], F32)
        zrA = ppool.tile([128, NT], F32)
        thA = ppool.tile([128, NT], F32)
        ssA = ppool.tile([128, NT], F32)
        nrA = ppool.tile([128, NT], F32)
        rnA = ppool.tile([128, NT], F32)
        facA = ppool.tile([128, NT], F32)
        # z = max(x_t, 1+eps); theta = ln(z + sqrt(z^2-1))
        xt_view = xall[:].rearrange("p (t c) -> p t c", c=513)[:, :, 0:1]
        nc.vector.tensor_scalar_max(zA[:], xt_view, 1.0 + EPS)
        nc.vector.tensor_mul(z2A[:], zA[:], zA[:])
        nc.vector.tensor_scalar_add(z2A[:], z2A[:], -1.0)
        nc.scalar.activation(rA[:], z2A[:], AF.Sqrt)
        nc.vector.tensor_add(zrA[:], zA[:], rA[:])
        nc.scalar.activation(thA[:], zrA[:], AF.Ln)
        # nrm = max(||x_s||, eps); fac = theta / nrm
        xs_view = xall[:].rearrange("p (t c) -> p t c", c=513)[:, :, 1:513]
        for g in range(4):
            nc.vector.tensor_mul(
                sqall[:].rearrange("p (t c) -> p t c", c=512),
                xs_view[:, g * 4:(g + 1) * 4], xs_view[:, g * 4:(g + 1) * 4],
            )
            nc.vector.reduce_sum(
                ssA[:, g * 4:(g + 1) * 4],
                sqall[:].rearrange("p (t c) -> p t c", c=512),
                axis=mybir.AxisListType.X,
            )
        nc.vector.tensor_scalar_max(nrA[:], ssA[:], EPS * EPS)
        nc.scalar.activation(nrA[:], nrA[:], AF.Sqrt)
        nc.vector.reciprocal(rnA[:], nrA[:])
        nc.vector.tensor_mul(facA[:], thA[:], rnA[:])

        # ---- stage A2+B1: transpose x_eu via diag matmul, then QKV ----
        for tt in range(NT):
            # x_euT chunk = xs_chunk.T @ diag(fac)
            diag_t = wpool.tile([128, 128], F32, tag="diag", bufs=2)
            nc.vector.tensor_mul(diag_t[:], ident[:], facA[:, tt:tt + 1].to_broadcast((128, 128)))
            xe_ps = pspool.tile([128, 512], F32, tag="misc")
            for c in range(4):
                nc.tensor.matmul(
                    xe_ps[:, c * 128:(c + 1) * 128],
                    lhsT=xall[:, tt * 513 + 1 + c * 128:tt * 513 + 1 + (c + 1) * 128],
                    rhs=diag_t[:],
                    start=True,
                    stop=True,
                )
            dst = xeT[tt % 2][:, (tt // 2) * 512:(tt // 2) * 512 + 512]
            if tt % 2 == 0:
                nc.vector.tensor_copy(dst, xe_ps[:])
            else:
                nc.scalar.copy(dst, xe_ps[:])

            # QKV projection for this token tile
            qkv_ps = pspool.tile([128, 384], F32, tag="misc")
            for c in range(4):
                nc.tensor.matmul(
                    qkv_ps[:],
                    lhsT=xeT[tt % 2][:, (tt // 2) * 512 + c * 128:(tt // 2) * 512 + (c + 1) * 128],
                    rhs=wqkv[:, c * 384:(c + 1) * 384],
                    start=(c == 0),
                    stop=(c == 3),
                )
            qdst = qkvN[:, tt * 384:(tt + 1) * 384]
            if tt % 2 == 0:
                nc.scalar.copy(qdst, qkv_ps[:])
            else:
                nc.vector.tensor_copy(qdst, qkv_ps[:])

        # ---- stage B2: batched exp-map stats over all 16 tiles ----
        for g in range(2):
            for tt in range(8 * g, 8 * g + 8):
                nc.vector.tensor_mul(
                    sqall[:, (tt - 8 * g) * 256:(tt - 8 * g + 1) * 256],
                    qkvN[:, tt * 384:tt * 384 + 256],
                    qkvN[:, tt * 384:tt * 384 + 256],
                )
            nc.vector.reduce_sum(
                ss_all[:, g * 32:(g + 1) * 32],
                sqall[:].rearrange("p (g d) -> p g d", d=64),
                axis=mybir.AxisListType.X,
            )
        nc.vector.tensor_scalar_max(ss_all[:], ss_all[:], EPS * EPS)
        nc.scalar.activation(n_all[:], ss_all[:], AF.Sqrt)
        nc.vector.tensor_mul(m_all[:], n_all[:], hc[:, 128:192])
        nc.scalar.activation(e1_all[:], m_all[:], AF.Exp)
        nc.vector.reciprocal(e2_all[:], e1_all[:])
        nc.vector.tensor_add(u_all[:], e1_all[:], e2_all[:])
        nc.vector.tensor_sub(w_all[:], e1_all[:], e2_all[:])
        nc.vector.reciprocal(rn_all[:], m_all[:])
        nc.vector.tensor_mul(w_all[:], w_all[:], rn_all[:])
        nc.vector.tensor_mul(g_all[:], w_all[:], hc[:, 0:64])
        nc.vector.tensor_mul(tv_all[:], u_all[:], hc[:, 64:128])

        # ---- stage B3: assemble Qt/Kt, transpose into qkT; fill vh ----
        for tt in range(NT):
            qnat = wpool.tile([128, 260], F32, tag="qnat", bufs=2)
            for j in range(4):
                nc.vector.tensor_mul(
                    qnat[:, j * 65:j * 65 + 64],
                    qkvN[:, tt * 384 + j * 64:tt * 384 + (j + 1) * 64],
                    g_all[:, tt * 4 + j:tt * 4 + j + 1].to_broadcast((128, 64)),
                )
            tcols = qnat[:].rearrange("p (j c) -> p j c", c=65)[:, :, 64:65]
            nc.vector.tensor_copy(tcols, tv_all[:, tt * 4:tt * 4 + 4])

            tr_ps = pspool.tile([65, 512], F32, tag="misc")
            for j in range(4):
                nc.tensor.transpose(
                    tr_ps[:, j * 128:(j + 1) * 128], qnat[:, j * 65:(j + 1) * 65],
                    ident[:],
                )
            qk_dst = qkT[:].rearrange("p (j s) -> p j s", s=2048)[
                :, :, tt * 128:(tt + 1) * 128
            ]
            tr_src = tr_ps[:].rearrange("p (j s) -> p j s", s=128)
            if tt % 2 == 0:
                nc.vector.tensor_copy(qk_dst, tr_src)
            else:
                nc.scalar.copy(qk_dst, tr_src)

            v_dst = vh[:].rearrange("p (h t c) -> p h t c", h=2, c=65)[
                :, :, tt, 0:64
            ]
            v_src = qkvN[:, tt * 384 + 256:tt * 384 + 384].rearrange(
                "p (h c) -> p h c", h=2
            )
            if tt % 2 == 0:
                nc.scalar.copy(v_dst, v_src)
            else:
                nc.vector.tensor_copy(v_dst, v_src)

        # ---- stage C: attention per head, per q block ----
        for h in range(2):
            for qb in range(4):
                pv_ps = pspool.tile([65, 512], F32, tag="pv")
                nkt = 4 * qb + 4
                for p in range(nkt // 2):
                    s_ps = pspool.tile([128, 1024], F32, tag="sc")
                    expS = wpool.tile([128, 1024], F32, tag="expS", bufs=3)
                    for j in range(2):
                        kt = 2 * p + j
                        nc.tensor.matmul(
                            s_ps[:, j * 512:(j + 1) * 512],
                            lhsT=qkT[:, (2 + h) * 2048 + kt * 128:(2 + h) * 2048 + (kt + 1) * 128],
                            rhs=qkT[:, h * 2048 + qb * 512:h * 2048 + (qb + 1) * 512],
                            start=True,
                            stop=True,
                        )
                    nc.scalar.activation(expS[:], s_ps[:], AF.Exp)
                    for j in range(2):
                        d = 2 * p + j - 4 * qb
                        if d >= 0:
                            nc.vector.tensor_mul(
                                expS[:, j * 512:(j + 1) * 512],
                                expS[:, j * 512:(j + 1) * 512],
                                maskt[:, d * 512:(d + 1) * 512],
                            )
                    for j in range(2):
                        kt = 2 * p + j
                        nc.tensor.matmul(
                            pv_ps[:],
                            lhsT=vh[:, (h * NT + kt) * 65:(h * NT + kt + 1) * 65],
                            rhs=expS[:, j * 512:(j + 1) * 512],
                            start=(kt == 0),
                            stop=(kt == nkt - 1),
                        )
                recip = wpool.tile([1, 512], F32, tag="recip", bufs=2)
                nc.vector.reciprocal(recip[:], pv_ps[64:65, :])
                bc_ps = pspool.tile([64, 512], F32, tag="misc")
                nc.tensor.matmul(
                    bc_ps[:], lhsT=ones64[:], rhs=recip[:], start=True, stop=True
                )
                bc_sb = wpool.tile([64, 512], F32, tag="bcsb", bufs=2)
                nc.scalar.copy(bc_sb[:], bc_ps[:])
                nc.vector.tensor_mul(
                    outT[h * 64:(h + 1) * 64, qb * 512:(qb + 1) * 512],
                    pv_ps[0:64, :],
                    bc_sb[:],
                )

        # ---- stage D: W_o row shard ----
        for qc in range(NT):
            wo_ps = pspool.tile([128, 512], F32, tag="misc")
            nc.tensor.matmul(
                wo_ps[:], lhsT=outT[:, qc * 128:(qc + 1) * 128], rhs=wo_t[:],
                start=True, stop=True,
            )
            outF = wpool.tile([128, 512], F32, tag="outF", bufs=3)
            if qc % 2 == 0:
                nc.vector.tensor_copy(outF[:], wo_ps[:])
            else:
                nc.scalar.copy(outF[:], wo_ps[:])
            nc.gpsimd.dma_start(out_d[qc * 128:(qc + 1) * 128, :], outF[:])

    nc.finalize()
    return nc


def _host_inputs(x, W_q, W_k, W_v, W_o, log_abs_K):
    x = np.asarray(x, np.float32)
    W_q = np.asarray(W_q, np.float32)
    W_k = np.asarray(W_k, np.float32)
    W_v = np.asarray(W_v, np.float32)
    W_o = np.asarray(W_o, np.float32)
    log_abs_K = np.asarray(log_abs_K, np.float32)

    abs_K = np.exp(log_abs_K.astype(np.float64))
    sc = np.sqrt(abs_K)
    c_sc = abs_K / np.sqrt(DH)

    masks = np.zeros((128, 2048), np.float32)
    jj = np.arange(512)
    pp = np.arange(128)[:, None]
    for d in range(4):
        masks[:, d * 512:(d + 1) * 512] = (jj >= pp + d * 128).astype(np.float32)
    ident = np.eye(128, dtype=np.float32)

    in_maps = []
    for core in range(NCORES):
        b = core // 4
        h0 = 2 * (core % 4)
        heads = [h0, h0 + 1]
        wq = np.concatenate([W_q[:, h * DH:(h + 1) * DH] for h in heads], axis=1)
        wk = np.concatenate([W_k[:, h * DH:(h + 1) * DH] for h in heads], axis=1)
        wv = np.concatenate([W_v[:, h * DH:(h + 1) * DH] for h in heads], axis=1)
        wqkv = np.concatenate([wq, wk, wv], axis=1)  # (512, 384)
        wo = np.concatenate([W_o[h * DH:(h + 1) * DH, :] for h in heads], axis=0)

        # per-column constants, pattern [qh0, qh1, kh0, kh1] x 16 tiles
        gq = [c_sc[h] / 2.0 for h in heads]
        gk = [-0.5, -0.5]
        tq = [c_sc[h] / (2.0 * sc[h]) for h in heads]
        tk = [1.0 / (2.0 * sc[h]) for h in heads]
        scn = [sc[h] for h in heads]
        gpat = np.array(gq + gk, np.float32)
        tpat = np.array(tq + tk, np.float32)
        spat = np.array(scn + scn, np.float32)
        hconst = np.zeros((128, 192), np.float32)
        hconst[:, 0:64] = np.tile(gpat, 16)[None, :]
        hconst[:, 64:128] = np.tile(tpat, 16)[None, :]
        hconst[:, 128:192] = np.tile(spat, 16)[None, :]

        in_maps.append(
            {
                "x": np.ascontiguousarray(x[b]),
                "wqkv": np.ascontiguousarray(wqkv),
                "wo": np.ascontiguousarray(wo),
                "masks": masks,
                "hconst": hconst,
                "ident": ident,
            }
        )
    return in_maps


def kernel(x, W_q, W_k, W_v, W_o, log_abs_K, _want_results=False, **_unused):
    in_maps = _host_inputs(x, W_q, W_k, W_v, W_o, log_abs_K)
    if "nc" not in _NC_CACHE:
        _NC_CACHE["nc"] = _emit_program()
    nc = _NC_CACHE["nc"]
    res = run_bass_kernel_spmd(nc, in_maps, list(range(NCORES)))
    out = np.zeros((B, S, D), np.float32)
    for core in range(NCORES):
        out[core // 4] += np.asarray(res.results[core]["out"])
    if _want_results:
        return out, res
    return out



# revision 2
# speedup vs baseline: 2.8600x; 2.8600x over previous
"""Lorentz per-head causal attention on 8 trn2 NeuronCores.

Sharding: core c -> batch b=c//4, heads {2*(c%4), 2*(c%4)+1}.
W_q/W_k/W_v column-sharded. The final W_o projection runs on host
(cheap [4096,512]@[512,512] f32 matmul) so each core only returns its
2 heads' attention output [128, 2048] fp16 (feature-major) -- this
minimizes host<->device transfer, which dominates wall time on the
axon-tunneled setup.

All DRAM I/O is fp16 with SWDGE cast-DMA to/from f32 SBUF; on-device
compute is f32 throughout. The causal mask tile and the 128x128
identity are generated on device (gpsimd affine_select), the per-head
constants are sent as one [1,192] f32 row and broadcast via a K=1
matmul. x is sent as x_s only ([2048,512]); the time component is
recomputed on device from the manifold constraint x_t=sqrt(1+|x_s|^2),
so theta = ln(sqrt(1+ss) + nrm).

Per-core kernel (all compute in f32):
  A: log-map x -> x_eu, transposed into [D,S] layout via per-token-tile
     matmuls against diag(theta/nrm) (fuses the scaling with the transpose).
  B: QKV projection [S,384] (2 heads x Q,K,V); batched exp-map stats;
     assemble Lorentz-lifted Qt=[c*f*Q, c*t], Kt=[-f*K, t] in [65,S] layout
     via PE transposes. V kept token-major with a ones column appended so
     the PV matmul also produces the softmax denominator for free.
  C: per head, per 512-wide q block: scoresT[k,q] matmuls (K=65), exp on
     ACT over [128,1024] pairs, causal masks (multiplicative, device-built)
     on diagonal tiles only, PV accumulation in PSUM [65,512]; normalize
     by broadcasting 1/denom with a K=1 ones matmul.
  D: cast-DMA outT [128,2048] f32 -> fp16 DRAM (feature-major).
Softmax skips max-subtraction: scores = abs_K*(qt*kt - qs.ks)/8 are O(1)
for these inputs (verified < 10), so exp cannot overflow.
"""
import sys

sys.path.insert(0, "/opt/trn_rl_repo")

from contextlib import ExitStack

import numpy as np

import concourse.bacc as bacc
import concourse.bass as bass
import concourse.mybir as mybir
from concourse.bass_utils import run_bass_kernel_spmd
from concourse.tile import TileContext

F32 = mybir.dt.float32
F16 = mybir.dt.float16
AF = mybir.ActivationFunctionType
ALU = mybir.AluOpType

B, S, D, H, DH = 2, 2048, 512, 8, 64
EPS = 1e-7
NT = S // 128  # 16 token tiles
NCORES = 8

_NC_CACHE = {}


def _emit_program():
    nc = bacc.Bacc(None)
    x_in = nc.declare_dram_parameter("x", [S, D], F16, isOutput=False)
    wqkv_in = nc.declare_dram_parameter("wqkv", [D, 384], F16, isOutput=False)
    hc_in = nc.declare_dram_parameter("hconst", [1, 192], F32, isOutput=False)
    out_d = nc.declare_dram_parameter("out", [128, S], F16, isOutput=True)

    with TileContext(nc) as tc, ExitStack() as ctx:
        cpool = ctx.enter_context(tc.tile_pool(name="consts", bufs=1))
        ppool = ctx.enter_context(tc.tile_pool(name="persist", bufs=1))
        wpool = ctx.enter_context(tc.tile_pool(name="work", bufs=3))
        pspool = ctx.enter_context(tc.tile_pool(name="ps", bufs=2, space="PSUM"))

        # ---- constants ----
        wqkv = cpool.tile([128, 4 * 384], F32)
        for c in range(4):
            nc.gpsimd.dma_start(
                wqkv[:, c * 384:(c + 1) * 384], wqkv_in[c * 128:(c + 1) * 128, :]
            )
        ones64 = cpool.tile([1, 64], F32)
        nc.vector.memset(ones64[:], 1.0)
        ones128 = cpool.tile([1, 128], F32)
        nc.vector.memset(ones128[:], 1.0)
        # identity (for PE transposes / diag matmuls), device-built
        ident = cpool.tile([128, 128], F32)
        nc.gpsimd.memset(ident[:], 0.0)
        nc.gpsimd.affine_select(
            out=ident[:], in_=ident[:], compare_op=ALU.not_equal, fill=1.0,
            base=0, pattern=[[-1, 128]], channel_multiplier=1,
        )
        # causal mask tiles: maskt[kk, d*512 + qq] = (qq >= kk + d*128)
        maskt = cpool.tile([128, 2048], F32)
        nc.gpsimd.memset(maskt[:], 1.0)
        for d in range(4):
            nc.gpsimd.affine_select(
                out=maskt[:, d * 512:(d + 1) * 512],
                in_=maskt[:, d * 512:(d + 1) * 512],
                compare_op=ALU.is_ge, fill=0.0,
                base=-d * 128, pattern=[[1, 512]], channel_multiplier=-1,
            )
        # per-head constants broadcast [1,192] -> [128,192]
        hc1 = cpool.tile([1, 192], F32)
        nc.gpsimd.dma_start(hc1[:], hc_in[:])
        hc_ps = pspool.tile([128, 192], F32, tag="misc")
        nc.tensor.matmul(hc_ps[:], lhsT=ones128[:], rhs=hc1[:], start=True, stop=True)
        hc = cpool.tile([128, 192], F32)
        nc.scalar.copy(hc[:], hc_ps[:])

        # ---- persistent intermediates ----
        # x_euT, per-tt chunk layout: tile[tt%2][:, (tt//2)*512 + c*128]
        xeTa = ppool.tile([128, 8 * 512], F32)
        xeTb = ppool.tile([128, 8 * 512], F32)
        xeT = [xeTa, xeTb]
        # [Qt_h0 | Qt_h1 | Kt_h0 | Kt_h1], each [65, 2048]
        qkT = ppool.tile([65, 4 * 2048], F32)
        # V-hat per head: NT groups of 65 cols, col 64 stays 1.0
        vh = ppool.tile([128, 2 * NT * 65], F32)
        nc.gpsimd.memset(vh[:], 1.0)
        qkvN = ppool.tile([128, NT * 384], F32)
        outT = ppool.tile([128, 4 * 512], F32)
        sqall = ppool.tile([128, 2048], F32)
        ss_all = ppool.tile([128, 64], F32)
        n_all = ppool.tile([128, 64], F32)
        m_all = ppool.tile([128, 64], F32)
        e1_all = ppool.tile([128, 64], F32)
        e2_all = ppool.tile([128, 64], F32)
        u_all = ppool.tile([128, 64], F32)
        w_all = ppool.tile([128, 64], F32)
        rn_all = ppool.tile([128, 64], F32)
        g_all = ppool.tile([128, 64], F32)
        tv_all = ppool.tile([128, 64], F32)

        # ---- stage A: batched log-map stats ----
        xall = ppool.tile([128, NT * 512], F32)
        nc.gpsimd.dma_start(
            xall[:].rearrange("p (t c) -> p t c", c=512),
            x_in[:].rearrange("(t p) c -> p t c", p=128),
        )
        zA = ppool.tile([128, NT], F32)
        z2A = ppool.tile([128, NT], F32)
        zrA = ppool.tile([128, NT], F32)
        thA = ppool.tile([128, NT], F32)
        ssA = ppool.tile([128, NT], F32)
        nrA = ppool.tile([128, NT], F32)
        rnA = ppool.tile([128, NT], F32)
        facA = ppool.tile([128, NT], F32)
        # ss = |x_s|^2 ; nrm = max(sqrt(ss), eps)
        xs_view = xall[:].rearrange("p (t c) -> p t c", c=512)
        for g in range(4):
            nc.vector.tensor_mul(
                sqall[:].rearrange("p (t c) -> p t c", c=512),
                xs_view[:, g * 4:(g + 1) * 4], xs_view[:, g * 4:(g + 1) * 4],
            )
            nc.vector.reduce_sum(
                ssA[:, g * 4:(g + 1) * 4],
                sqall[:].rearrange("p (t c) -> p t c", c=512),
                axis=mybir.AxisListType.X,
            )
        nc.vector.tensor_scalar_max(nrA[:], ssA[:], EPS * EPS)
        nc.scalar.activation(nrA[:], nrA[:], AF.Sqrt)
        nc.vector.reciprocal(rnA[:], nrA[:])
        # x_t = sqrt(1 + ss); theta = ln(x_t + nrm); fac = theta / nrm
        nc.vector.tensor_scalar_add(z2A[:], ssA[:], 1.0)
        nc.scalar.activation(zA[:], z2A[:], AF.Sqrt)
        nc.vector.tensor_add(zrA[:], zA[:], nrA[:])
        nc.scalar.activation(thA[:], zrA[:], AF.Ln)
        nc.vector.tensor_mul(facA[:], thA[:], rnA[:])

        # ---- stage A2+B1: transpose x_eu via diag matmul, then QKV ----
        for tt in range(NT):
            # x_euT chunk = xs_chunk.T @ diag(fac)
            diag_t = wpool.tile([128, 128], F32, tag="diag", bufs=2)
            nc.vector.tensor_mul(diag_t[:], ident[:], facA[:, tt:tt + 1].to_broadcast((128, 128)))
            xe_ps = pspool.tile([128, 512], F32, tag="misc")
            for c in range(4):
                nc.tensor.matmul(
                    xe_ps[:, c * 128:(c + 1) * 128],
                    lhsT=xall[:, tt * 512 + c * 128:tt * 512 + (c + 1) * 128],
                    rhs=diag_t[:],
                    start=True,
                    stop=True,
                )
            dst = xeT[tt % 2][:, (tt // 2) * 512:(tt // 2) * 512 + 512]
            if tt % 2 == 0:
                nc.vector.tensor_copy(dst, xe_ps[:])
            else:
                nc.scalar.copy(dst, xe_ps[:])

            # QKV projection for this token tile
            qkv_ps = pspool.tile([128, 384], F32, tag="misc")
            for c in range(4):
                nc.tensor.matmul(
                    qkv_ps[:],
                    lhsT=xeT[tt % 2][:, (tt // 2) * 512 + c * 128:(tt // 2) * 512 + (c + 1) * 128],
                    rhs=wqkv[:, c * 384:(c + 1) * 384],
                    start=(c == 0),
                    stop=(c == 3),
                )
            qdst = qkvN[:, tt * 384:(tt + 1) * 384]
            if tt % 2 == 0:
                nc.scalar.copy(qdst, qkv_ps[:])
            else:
                nc.vector.tensor_copy(qdst, qkv_ps[:])

        # ---- stage B2: batched exp-map stats over all 16 tiles ----
        for g in range(2):
            for tt in range(8 * g, 8 * g + 8):
                nc.vector.tensor_mul(
                    sqall[:, (tt - 8 * g) * 256:(tt - 8 * g + 1) * 256],
                    qkvN[:, tt * 384:tt * 384 + 256],
                    qkvN[:, tt * 384:tt * 384 + 256],
                )
            nc.vector.reduce_sum(
                ss_all[:, g * 32:(g + 1) * 32],
                sqall[:].rearrange("p (g d) -> p g d", d=64),
                axis=mybir.AxisListType.X,
            )
        nc.vector.tensor_scalar_max(ss_all[:], ss_all[:], EPS * EPS)
        nc.scalar.activation(n_all[:], ss_all[:], AF.Sqrt)
        nc.vector.tensor_mul(m_all[:], n_all[:], hc[:, 128:192])
        nc.scalar.activation(e1_all[:], m_all[:], AF.Exp)
        nc.vector.reciprocal(e2_all[:], e1_all[:])
        nc.vector.tensor_add(u_all[:], e1_all[:], e2_all[:])
        nc.vector.tensor_sub(w_all[:], e1_all[:], e2_all[:])
        nc.vector.reciprocal(rn_all[:], m_all[:])
        nc.vector.tensor_mul(w_all[:], w_all[:], rn_all[:])
        nc.vector.tensor_mul(g_all[:], w_all[:], hc[:, 0:64])
        nc.vector.tensor_mul(tv_all[:], u_all[:], hc[:, 64:128])

        # ---- stage B3: assemble Qt/Kt, transpose into qkT; fill vh ----
        for tt in range(NT):
            qnat = wpool.tile([128, 260], F32, tag="qnat", bufs=2)
            for j in range(4):
                nc.vector.tensor_mul(
                    qnat[:, j * 65:j * 65 + 64],
                    qkvN[:, tt * 384 + j * 64:tt * 384 + (j + 1) * 64],
                    g_all[:, tt * 4 + j:tt * 4 + j + 1].to_broadcast((128, 64)),
                )
            tcols = qnat[:].rearrange("p (j c) -> p j c", c=65)[:, :, 64:65]
            nc.vector.tensor_copy(tcols, tv_all[:, tt * 4:tt * 4 + 4])

            tr_ps = pspool.tile([65, 512], F32, tag="misc")
            for j in range(4):
                nc.tensor.transpose(
                    tr_ps[:, j * 128:(j + 1) * 128], qnat[:, j * 65:(j + 1) * 65],
                    ident[:],
                )
            qk_dst = qkT[:].rearrange("p (j s) -> p j s", s=2048)[
                :, :, tt * 128:(tt + 1) * 128
            ]
            tr_src = tr_ps[:].rearrange("p (j s) -> p j s", s=128)
            if tt % 2 == 0:
                nc.vector.tensor_copy(qk_dst, tr_src)
            else:
                nc.scalar.copy(qk_dst, tr_src)

            v_dst = vh[:].rearrange("p (h t c) -> p h t c", h=2, c=65)[
                :, :, tt, 0:64
            ]
            v_src = qkvN[:, tt * 384 + 256:tt * 384 + 384].rearrange(
                "p (h c) -> p h c", h=2
            )
            if tt % 2 == 0:
                nc.scalar.copy(v_dst, v_src)
            else:
                nc.vector.tensor_copy(v_dst, v_src)

        # ---- stage C: attention per head, per q block ----
        for h in range(2):
            for qb in range(4):
                pv_ps = pspool.tile([65, 512], F32, tag="pv")
                nkt = 4 * qb + 4
                for p in range(nkt // 2):
                    s_ps = pspool.tile([128, 1024], F32, tag="sc")
                    expS = wpool.tile([128, 1024], F32, tag="expS", bufs=3)
                    for j in range(2):
                        kt = 2 * p + j
                        nc.tensor.matmul(
                            s_ps[:, j * 512:(j + 1) * 512],
                            lhsT=qkT[:, (2 + h) * 2048 + kt * 128:(2 + h) * 2048 + (kt + 1) * 128],
                            rhs=qkT[:, h * 2048 + qb * 512:h * 2048 + (qb + 1) * 512],
                            start=True,
                            stop=True,
                        )
                    nc.scalar.activation(expS[:], s_ps[:], AF.Exp)
                    for j in range(2):
                        d = 2 * p + j - 4 * qb
                        if d >= 0:
                            nc.vector.tensor_mul(
                                expS[:, j * 512:(j + 1) * 512],
                                expS[:, j * 512:(j + 1) * 512],
                                maskt[:, d * 512:(d + 1) * 512],
                            )
                    for j in range(2):
                        kt = 2 * p + j
                        nc.tensor.matmul(
                            pv_ps[:],
                            lhsT=vh[:, (h * NT + kt) * 65:(h * NT + kt + 1) * 65],
                            rhs=expS[:, j * 512:(j + 1) * 512],
                            start=(kt == 0),
                            stop=(kt == nkt - 1),
                        )
                recip = wpool.tile([1, 512], F32, tag="recip", bufs=2)
                nc.vector.reciprocal(recip[:], pv_ps[64:65, :])
                bc_ps = pspool.tile([64, 512], F32, tag="misc")
                nc.tensor.matmul(
                    bc_ps[:], lhsT=ones64[:], rhs=recip[:], start=True, stop=True
                )
                bc_sb = wpool.tile([64, 512], F32, tag="bcsb", bufs=2)
                nc.scalar.copy(bc_sb[:], bc_ps[:])
                nc.vector.tensor_mul(
                    outT[h * 64:(h + 1) * 64, qb * 512:(qb + 1) * 512],
                    pv_ps[0:64, :],
                    bc_sb[:],
                )

        # ---- stage D: cast-DMA attnV out (feature-major) ----
        nc.gpsimd.dma_start(out_d[:], outT[:])

    nc.finalize()
    return nc


def _host_inputs(x, W_q, W_k, W_v, W_o, log_abs_K):
    x = np.asarray(x, np.float32)
    W_q = np.asarray(W_q, np.float32)
    W_k = np.asarray(W_k, np.float32)
    W_v = np.asarray(W_v, np.float32)
    log_abs_K = np.asarray(log_abs_K, np.float32)

    abs_K = np.exp(log_abs_K.astype(np.float64))
    sc = np.sqrt(abs_K)
    c_sc = abs_K / np.sqrt(DH)

    xs16 = [np.ascontiguousarray(x[b, :, 1:]).astype(np.float16) for b in range(B)]

    in_maps = []
    for core in range(NCORES):
        b = core // 4
        h0 = 2 * (core % 4)
        heads = [h0, h0 + 1]
        wq = np.concatenate([W_q[:, h * DH:(h + 1) * DH] for h in heads], axis=1)
        wk = np.concatenate([W_k[:, h * DH:(h + 1) * DH] for h in heads], axis=1)
        wv = np.concatenate([W_v[:, h * DH:(h + 1) * DH] for h in heads], axis=1)
        wqkv = np.concatenate([wq, wk, wv], axis=1).astype(np.float16)  # (512, 384)

        # per-column constants, pattern [qh0, qh1, kh0, kh1] x 16 tiles
        gq = [c_sc[h] / 2.0 for h in heads]
        gk = [-0.5, -0.5]
        tq = [c_sc[h] / (2.0 * sc[h]) for h in heads]
        tk = [1.0 / (2.0 * sc[h]) for h in heads]
        scn = [sc[h] for h in heads]
        gpat = np.array(gq + gk, np.float32)
        tpat = np.array(tq + tk, np.float32)
        spat = np.array(scn + scn, np.float32)
        hconst = np.zeros((1, 192), np.float32)
        hconst[0, 0:64] = np.tile(gpat, 16)
        hconst[0, 64:128] = np.tile(tpat, 16)
        hconst[0, 128:192] = np.tile(spat, 16)

        in_maps.append(
            {
                "x": xs16[b],
                "wqkv": wqkv,
                "hconst": hconst,
            }
        )
    return in_maps


def kernel(x, W_q, W_k, W_v, W_o, log_abs_K, _want_results=False, **_unused):
    in_maps = _host_inputs(x, W_q, W_k, W_v, W_o, log_abs_K)
    if "nc" not in _NC_CACHE:
        _NC_CACHE["nc"] = _emit_program()
    nc = _NC_CACHE["nc"]
    res = run_bass_kernel_spmd(nc, in_maps, list(range(NCORES)))
    W_o = np.asarray(W_o, np.float32)
    out = np.empty((B, S, D), np.float32)
    for b in range(B):
        attnV = np.concatenate(
            [np.asarray(res.results[4 * b + j]["out"]) for j in range(4)], axis=0
        )  # [512 feats, 2048 tokens] fp16, head-major rows
        out[b] = attnV.T.astype(np.float32) @ W_o
    if _want_results:
        return out, res
    return out
